# revision 1
# baseline (speedup 1.0000x reference)
"""Trainium2 Bass kernel for 2-layer LSTM (H=16) time-series predictor.

Model (reference): x:[B,T] -> per-t scalar input into LSTMCell1(1->16) ->
LSTMCell2(16->16), teacher-forced over T steps, then head(h2)=fc2(leaky(fc1(h2)))
produces out[:,0]; 32 autoregressive steps feed head output back as input.
Output [B, 33].

Sharding: data-parallel over batch across 8 cores (256 batch each), weights
replicated. Per-core layout: hidden dim on partitions, batch on the free dim.

Hardware constraints that shape the design:
  * every vector/scalar instruction needs ALL operands on the same partition
    range -> every elementwise tensor (c, h, sigmoid/tanh outputs) lives on
    partition window 0:32 ([layer1; layer2] stacked), and the 4 gate types are
    separated along the PSUM *free* dim instead of partitions;
  * a matmul writes one [M<=32-aligned, N<=512] PSUM block -> 4 matmuls per
    step, one per gate type (i, f, o, g), each [32, 256];
  * layer2 lags one step so both layers' gates use the same h1 and one shared
    rhs: a rotating hx buffer [h1(16); h2(16); x_t(1); ones(1)], whose x/ones
    rows are DMA-prefilled straight from DRAM several steps ahead.
"""

import numpy as np

import concourse.bass as bass
import concourse.tile as tile
from concourse import bacc, mybir
from concourse.bass_utils import run_bass_kernel_spmd

F32 = mybir.dt.float32
AF = mybir.ActivationFunctionType

H = 16
B = 2048
T = 2048
FUT = 32
NCORES = 8
BC = B // NCORES  # 256 batch per core
NHX = 2           # rotation depth of the hx rhs buffers

# torch gate row order in the 4H weight matrices: i, f, g, o
_G = {"i": slice(0, H), "f": slice(H, 2 * H), "g": slice(2 * H, 3 * H), "o": slice(3 * H, 4 * H)}
# our gate order along the psum free dim / lhsT column blocks
_ORDER = ["i", "f", "o", "g"]


def _pack_weights(W_ih1, W_hh1, b_ih1, b_hh1, W_ih2, W_hh2, b_ih2, b_hh2,
                  fc1_w, fc1_b, fc2_w, fc2_b):
    b1 = b_ih1 + b_hh1  # [64]
    b2 = b_ih2 + b_hh2

    # main loop lhsTs; column block k (32 wide) = gate _ORDER[k], [l1(16)|l2(16)].
    # main_h rows = [h1(16); h2(16)]; main_x rows = [x(1); ones(1)].
    main_h = np.zeros((32, 128), np.float32)
    main_x = np.zeros((2, 128), np.float32)
    for k, gn in enumerate(_ORDER):
        c0 = 32 * k
        main_h[0:16, c0:c0 + 16] = W_hh1[_G[gn], :].T      # h1 -> layer1 gate
        main_h[0:16, c0 + 16:c0 + 32] = W_ih2[_G[gn], :].T  # h1 -> layer2 gate
        main_h[16:32, c0 + 16:c0 + 32] = W_hh2[_G[gn], :].T  # h2 -> layer2 gate
        main_x[0, c0:c0 + 16] = W_ih1[_G[gn], 0]            # x  -> layer1 gate
        main_x[1, c0:c0 + 16] = b1[_G[gn]]
        main_x[1, c0 + 16:c0 + 32] = b2[_G[gn]]

    # rollout layer1: split into K=1 part (input o) and K=17 part (h1 + bias)
    ro1x = np.zeros((1, 64), np.float32)   # columns: 4 gate blocks of 16
    ro1h = np.zeros((17, 64), np.float32)
    ro2h1 = np.zeros((17, 64), np.float32)  # h1 + bias part of layer2
    ro2h2 = np.zeros((16, 64), np.float32)  # h2 part of layer2
    for k, gn in enumerate(_ORDER):
        c0 = 16 * k
        ro1x[0, c0:c0 + 16] = W_ih1[_G[gn], 0]
        ro1h[0:16, c0:c0 + 16] = W_hh1[_G[gn], :].T
        ro1h[16, c0:c0 + 16] = b1[_G[gn]]
        ro2h1[0:16, c0:c0 + 16] = W_ih2[_G[gn], :].T
        ro2h1[16, c0:c0 + 16] = b2[_G[gn]]
        ro2h2[0:16, c0:c0 + 16] = W_hh2[_G[gn], :].T

    ro_fc1 = np.zeros((17, 8), np.float32)
    ro_fc1[0:16] = fc1_w.T  # fc1_w [8,16]
    ro_fc1[16] = fc1_b

    # M=1 matmuls misbehave on HW — replicate the fc2 column into M=8 and
    # read row 0 of the result instead
    ro_fc2 = np.zeros((9, 8), np.float32)
    ro_fc2[0:8] = fc2_w.T  # fc2_w [1,8]
    ro_fc2[8] = fc2_b

    return dict(main_h=main_h, main_x=main_x, ro1x=ro1x, ro1h=ro1h,
                ro2h1=ro2h1, ro2h2=ro2h2, ro_fc1=ro_fc1, ro_fc2=ro_fc2)


def _pack_x(x_core, t_steps):
    """x_core [BC, t] -> [t+1, 2, BC]: per step a [x_t; 1] pair (last x row 0)."""
    xe = np.ones((t_steps + 1, 2, x_core.shape[0]), np.float32)
    xe[:, 0, :] = 0.0
    xe[:t_steps, 0, :] = x_core.T.astype(np.float32)
    return xe


# ---------------------------------------------------------------------------
# device kernel
# ---------------------------------------------------------------------------

def _build(t_steps=T, fut=FUT, bc=BC, dbg=False, loop_steps=None):
    # loop_steps < t_steps runs fewer recurrence steps with identical I/O
    # sizes — for isolating device time via wall-clock deltas (output is
    # mathematically meaningless in that mode)
    nc = bacc.Bacc("TRN2", target_bir_lowering=False)
    dbg_d = {}
    if dbg == 3:
        for name, p in [("d_zt", 9), ("d_z", 8), ("d_ops", 8)]:
            dbg_d[name] = nc.dram_tensor(name, [p, bc], F32, kind="ExternalOutput")
        dbg_d["d_osb"] = nc.dram_tensor("d_osb", [1, fut + 1, bc], F32, kind="ExternalOutput")
    elif dbg:
        for name, p in [("d_h1e", 17), ("d_h2e", 17), ("d_rc1", 16), ("d_rc2", 16)]:
            dbg_d[name] = nc.dram_tensor(name, [p, bc], F32, kind="ExternalOutput")

    xe_d = nc.dram_tensor("xe", [t_steps + 1, 2, bc], F32, kind="ExternalInput")
    w_d = {}
    for name, shape in [("main_h", [32, 128]), ("main_x", [2, 128]),
                        ("ro1x", [1, 64]), ("ro1h", [17, 64]),
                        ("ro2h1", [17, 64]), ("ro2h2", [16, 64]),
                        ("ro_fc1", [17, 8]), ("ro_fc2", [9, 8])]:
        w_d[name] = nc.dram_tensor(name, shape, F32, kind="ExternalInput")
    out_d = nc.dram_tensor("out", [fut + 1, bc], F32, kind="ExternalOutput")

    with tile.TileContext(nc) as tc:
        consts = tc.alloc_tile_pool(name="consts", bufs=1)
        states = tc.alloc_tile_pool(name="states", bufs=1)
        work = tc.alloc_tile_pool(name="work", bufs=3)
        xst = tc.alloc_tile_pool(name="xst", bufs=8)
        # main psum (4 banks) and rollout psum (4 banks) stay disjoint for the
        # whole kernel: recycling banks across pools while late main-loop ACT
        # reads are in flight corrupts results (PE-write/engine-read same-bank
        # hazard)
        psum = tc.alloc_tile_pool(name="psum", bufs=2, space="PSUM")
        psro = tc.alloc_tile_pool(name="psro", bufs=1, space="PSUM")

        w_sb = {}
        for name, t_d in w_d.items():
            w_sb[name] = consts.tile(list(t_d.shape), F32, tag=name, name=name)
            nc.sync.dma_start(out=w_sb[name], in_=t_d[:])

        # rotating rhs buffers: [h1(0:16); h2(16:32)]
        hx = []
        for q in range(NHX):
            hq = states.tile([32, bc], F32, tag=f"hx{q}", name=f"hx{q}")
            nc.vector.memset(hq, 0.0)
            hx.append(hq)
        cc = states.tile([32, bc], F32, tag="cc")   # [c1; c2]
        nc.vector.memset(cc, 0.0)

        wmh, wmx = w_sb["main_h"], w_sb["main_x"]

        # rollout state tiles (declared early; layer-1 snapshots are taken
        # between main-loop steps T-1 and T)
        h1e = states.tile([17, bc], F32, tag="h1e")  # h1 | ones
        h2e = states.tile([17, bc], F32, tag="h2e")  # h2 | ones
        rc1 = states.tile([16, bc], F32, tag="rc1")
        rc2 = states.tile([16, bc], F32, tag="rc2")

        # ---------------- main teacher-forced loop ----------------
        def body(j):
            cur = hx[j % NHX]
            nxt = hx[(j + 1) % NHX]
            xs = xst.tile([2, bc], F32, tag="xs")
            nc.sync.dma_start(out=xs, in_=xe_d[j])

            g = psum.tile([32, 4, bc], F32, tag="g")  # free: gate-type x batch
            for k in range(4):
                # x+bias then h, closing each accumulation group before the
                # next opens (concurrent groups in one psum zero region are
                # illegal)
                nc.tensor.matmul(g[:, k, :], wmx[:, 32 * k:32 * k + 32], xs,
                                 start=True, stop=False)
                nc.tensor.matmul(g[:, k, :], wmh[:, 32 * k:32 * k + 32], cur,
                                 start=False, stop=True)

            sif = work.tile([32, 3, bc], F32, tag="sif")
            nc.scalar.activation(sif, g[:, 0:3, :], AF.Sigmoid)
            tg = work.tile([32, bc], F32, tag="tg")
            nc.scalar.activation(tg, g[:, 3, :], AF.Tanh)

            # j==0: layer-1 half only (layer-2 gates are not yet valid).
            # j==t_steps: full window (base-16 slices are illegal); the
            # layer-1 results of this step are junk but harmless — rc1/h1e
            # snapshot c1(T-1)/h1(T-1) before this step's writes land.
            s0, s1 = (0, 16) if j == 0 else (0, 32)
            m1 = work.tile([32, bc], F32, tag="m1")
            m2 = work.tile([32, bc], F32, tag="m2")
            tc_ = work.tile([32, bc], F32, tag="tc")
            nc.vector.tensor_mul(m1[s0:s1], sif[s0:s1, 1, :], cc[s0:s1])
            nc.vector.tensor_mul(m2[s0:s1], sif[s0:s1, 0, :], tg[s0:s1])
            nc.vector.tensor_add(cc[s0:s1], m1[s0:s1], m2[s0:s1])
            nc.scalar.activation(tc_[s0:s1], cc[s0:s1], AF.Tanh)
            nc.vector.tensor_mul(nxt[s0:s1], sif[s0:s1, 2, :], tc_[s0:s1])

        n_loop = t_steps if loop_steps is None else loop_steps
        for j in range(n_loop):
            body(j)
        # snapshot layer-1 state before the final (layer-2-only) step clobbers it
        nc.scalar.copy(h1e[0:16], hx[n_loop % NHX][0:16])   # h1(T-1)
        nc.scalar.copy(rc1, cc[0:16])                        # c1(T-1)
        body(n_loop)

        # ---------------- rollout ----------------
        ot = states.tile([1, bc], F32, tag="ot")     # current head output
        zt = states.tile([9, bc], F32, tag="zt")     # leaky(fc1) | ones
        out_sb = states.tile([1, fut + 1, bc], F32, tag="out_sb")
        # ones rows (memset can't start at partition 16/8 — DMA from xe ones row)
        nc.sync.dma_start(out=h1e[16:17, :], in_=xe_d[n_loop, 1:2])
        nc.sync.dma_start(out=h2e[16:17, :], in_=xe_d[n_loop, 1:2])
        nc.sync.dma_start(out=zt[8:9, :], in_=xe_d[n_loop, 1:2])

        nc.sync.dma_start(out=h2e[0:16, :], in_=hx[(n_loop + 1) % NHX][16:32, :])  # h2(T-1), repartition
        nc.sync.dma_start(out=rc2[:], in_=cc[16:32, :])

        if dbg == 1:
            for name, t in [("d_h1e", h1e), ("d_h2e", h2e), ("d_rc1", rc1), ("d_rc2", rc2)]:
                nc.sync.dma_start(out=dbg_d[name][:], in_=t[:])

        last_ops = []

        def head(r):
            z = psro.tile([8, bc], F32, tag="roz")
            nc.tensor.matmul(z, w_sb["ro_fc1"], h2e, start=True, stop=True)
            zs = work.tile([8, bc], F32, tag="zs")
            nc.scalar.mul(zs, z, 0.2)
            nc.vector.tensor_max(zt[0:8], z, zs)  # leaky relu 0.2
            o_ps = psro.tile([8, bc], F32, tag="roo")
            nc.tensor.matmul(o_ps, w_sb["ro_fc2"], zt, start=True, stop=True)
            last_ops[:] = [o_ps]
            nc.scalar.copy(out_sb[:, r, :], o_ps[0:1])
            if r <= fut - 1:
                nc.scalar.copy(ot, o_ps[0:1])

        def ro_cell(mms, rc, h_out):
            gr = psro.tile([16, 4, bc], F32, tag="rog")
            for k in range(4):
                for i, (lhsT, rhs) in enumerate(mms):
                    nc.tensor.matmul(gr[:, k, :], lhsT[:, 16 * k:16 * k + 16], rhs,
                                     start=(i == 0), stop=(i == len(mms) - 1))
            sifr = work.tile([16, 3, bc], F32, tag="sifr")
            nc.scalar.activation(sifr, gr[:, 0:3, :], AF.Sigmoid)
            tgr = work.tile([16, bc], F32, tag="tgr")
            nc.scalar.activation(tgr, gr[:, 3, :], AF.Tanh)
            a1 = work.tile([16, bc], F32, tag="a1")
            a2 = work.tile([16, bc], F32, tag="a2")
            tcr = work.tile([16, bc], F32, tag="tcr")
            nc.vector.tensor_mul(a1, sifr[:, 1, :], rc)
            nc.vector.tensor_mul(a2, sifr[:, 0, :], tgr)
            nc.vector.tensor_add(rc, a1, a2)
            nc.scalar.activation(tcr, rc, AF.Tanh)
            nc.vector.tensor_mul(h_out, sifr[:, 2, :], tcr)

        head(0)
        for r in range(fut):
            ro_cell([(w_sb["ro1x"], ot), (w_sb["ro1h"], h1e)], rc1, h1e[0:16])
            ro_cell([(w_sb["ro2h1"], h1e), (w_sb["ro2h2"], h2e[0:16])], rc2, h2e[0:16])
            head(r + 1)

        if dbg == 2:
            for name, t in [("d_h1e", h1e), ("d_h2e", h2e), ("d_rc1", rc1), ("d_rc2", rc2)]:
                nc.sync.dma_start(out=dbg_d[name][:], in_=t[:])
        if dbg == 3:
            nc.sync.dma_start(out=dbg_d["d_zt"][:], in_=zt[:])
            ops_sb = states.tile([8, bc], F32, tag="ops_sb")
            nc.scalar.copy(ops_sb, last_ops[0])
            nc.sync.dma_start(out=dbg_d["d_ops"][:], in_=ops_sb[:])
            nc.sync.dma_start(out=dbg_d["d_osb"][:], in_=out_sb[:])
            z2 = psro.tile([8, bc], F32, tag="roz")
            nc.tensor.matmul(z2, w_sb["ro_fc1"], h2e, start=True, stop=True)
            z2s = states.tile([8, bc], F32, tag="z2s")
            nc.scalar.copy(z2s, z2)
            nc.sync.dma_start(out=dbg_d["d_z"][:], in_=z2s[:])

        # keep the partition dim in the AP — integer-indexing it away breaks
        # Tile's subtile dependency tracking (the DMA then reads stale data)
        nc.sync.dma_start(out=out_d[:].rearrange("(o f) b -> o f b", o=1), in_=out_sb)

        for p_ in (psro, psum, xst, work, states, consts):
            p_.release()

    if not nc.is_finalized():
        nc.finalize()
    return nc


_CACHED = {}


def _get_nc(t_steps, fut, bc, loop_steps=None):
    key = (t_steps, fut, bc, loop_steps)
    if key not in _CACHED:
        _CACHED[key] = _build(t_steps, fut, bc, loop_steps=loop_steps)
    return _CACHED[key]


def kernel(x, W_ih1, W_hh1, b_ih1, b_hh1, W_ih2, W_hh2, b_ih2, b_hh2,
           fc1_w, fc1_b, fc2_w, fc2_b, future, _t_steps=None, _trace=False,
           _loop_steps=None):
    x = np.asarray(x, np.float32)
    fut = int(future)
    t_steps = int(_t_steps or x.shape[1])
    bc = x.shape[0] // NCORES

    w = _pack_weights(np.asarray(W_ih1, np.float32), np.asarray(W_hh1, np.float32),
                      np.asarray(b_ih1, np.float32), np.asarray(b_hh1, np.float32),
                      np.asarray(W_ih2, np.float32), np.asarray(W_hh2, np.float32),
                      np.asarray(b_ih2, np.float32), np.asarray(b_hh2, np.float32),
                      np.asarray(fc1_w, np.float32), np.asarray(fc1_b, np.float32),
                      np.asarray(fc2_w, np.float32), np.asarray(fc2_b, np.float32))

    nc = _get_nc(t_steps, fut, bc, _loop_steps)
    in_maps = []
    for c in range(NCORES):
        m = dict(w)
        m["xe"] = _pack_x(x[c * bc : (c + 1) * bc, :t_steps], t_steps)
        in_maps.append(m)

    res = run_bass_kernel_spmd(nc, in_maps, core_ids=list(range(NCORES)), trace=_trace)
    outs = [res.results[c]["out"] for c in range(NCORES)]  # each [fut+1, bc]
    full = np.concatenate(outs, axis=1).T  # [B, fut+1]
    kernel._last_exec_ns = res.exec_time_ns
    return np.ascontiguousarray(full.astype(np.float32))



# revision 3
# speedup vs baseline: 1004.0561x; 1004.0561x over previous
"""Trainium2 Bass kernel for 2-layer LSTM (H=16) time-series predictor.

Model (reference): x:[B,T] -> per-t scalar input into LSTMCell1(1->16) ->
LSTMCell2(16->16), teacher-forced over T steps, then head(h2)=fc2(leaky(fc1(h2)))
produces out[:,0]; 32 autoregressive steps feed head output back as input.
Output [B, 33].

Sharding: data-parallel over batch across 8 cores (256 batch each), weights
replicated. Per-core layout: hidden dim on partitions, batch on the free dim.

v2 vs the unrolled baseline: the 2048-step teacher-forced recurrence runs in a
hardware For_i loop (32 steps per iteration), so the program is ~1.5k
instructions instead of ~33k. Per step: 8 matmuls (per gate: K=1 x-part +
K=33 h-part with the bias folded in via a persistent ones row 32 of the rhs),
2+1 ACT ops, 4 DVE ops. x is DMA-prefetched one 32-step chunk per iteration
via a DynSlice on the loop register. The compiled jax/PJRT runner is cached
per-program so repeat calls skip the client-side BIR reverify.

Layer2 lags one step: step j computes h1^(j+1) (consuming x_j) and h2^(j)
(consuming h1^(j) from the rotating rhs buffer); one extra layer2-only step
at j=T finishes h2, with layer-1 state snapshotted just before it.
"""

import numpy as np

import concourse.bass as bass
import concourse.tile as tile
from concourse import bacc, bass2jax, mybir
from concourse.bass import ds

F32 = mybir.dt.float32
BF16 = mybir.dt.bfloat16
AF = mybir.ActivationFunctionType

H = 16
B = 2048
T = 2048
FUT = 32
NCORES = 8
BC = B // NCORES  # 256 batch per core
U = 32            # recurrence steps per hardware-loop iteration

# torch gate row order in the 4H weight matrices: i, f, g, o
_G = {"i": slice(0, H), "f": slice(H, 2 * H), "g": slice(2 * H, 3 * H), "o": slice(3 * H, 4 * H)}
# our gate order along the psum free dim / lhsT column blocks. f first so the
# f-sigmoid (and the c-update chain behind it) can start after one matmul.
_ORDER = ["f", "i", "o", "g"]


def _pack_weights(W_ih1, W_hh1, b_ih1, b_hh1, W_ih2, W_hh2, b_ih2, b_hh2,
                  fc1_w, fc1_b, fc2_w, fc2_b):
    b1 = b_ih1 + b_hh1  # [64]
    b2 = b_ih2 + b_hh2

    # main loop lhsT; column block k (32 wide) = gate _ORDER[k], [l1(16)|l2(16)].
    # mh rows = [h1(16); h2(16); ones(1)->bias; x(1)] — one matmul per gate.
    mh = np.zeros((34, 128), np.float32)
    for k, gn in enumerate(_ORDER):
        c0 = 32 * k
        mh[0:16, c0:c0 + 16] = W_hh1[_G[gn], :].T       # h1 -> layer1 gate
        mh[0:16, c0 + 16:c0 + 32] = W_ih2[_G[gn], :].T  # h1 -> layer2 gate
        mh[16:32, c0 + 16:c0 + 32] = W_hh2[_G[gn], :].T  # h2 -> layer2 gate
        mh[32, c0:c0 + 16] = b1[_G[gn]]
        mh[32, c0 + 16:c0 + 32] = b2[_G[gn]]
        mh[33, c0:c0 + 16] = W_ih1[_G[gn], 0]           # x -> layer1 gate

    # rollout layer1: split into K=1 part (input o) and K=17 part (h1 + bias)
    ro1x = np.zeros((1, 64), np.float32)   # columns: 4 gate blocks of 16
    ro1h = np.zeros((17, 64), np.float32)
    ro2h1 = np.zeros((17, 64), np.float32)  # h1 + bias part of layer2
    ro2h2 = np.zeros((16, 64), np.float32)  # h2 part of layer2
    for k, gn in enumerate(_ORDER):
        c0 = 16 * k
        ro1x[0, c0:c0 + 16] = W_ih1[_G[gn], 0]
        ro1h[0:16, c0:c0 + 16] = W_hh1[_G[gn], :].T
        ro1h[16, c0:c0 + 16] = b1[_G[gn]]
        ro2h1[0:16, c0:c0 + 16] = W_ih2[_G[gn], :].T
        ro2h1[16, c0:c0 + 16] = b2[_G[gn]]
        ro2h2[0:16, c0:c0 + 16] = W_hh2[_G[gn], :].T

    ro_fc1 = np.zeros((17, 8), np.float32)
    ro_fc1[0:16] = fc1_w.T  # fc1_w [8,16]
    ro_fc1[16] = fc1_b

    # M=1 matmuls misbehave on HW — replicate the fc2 column into M=8 and
    # read row 0 of the result instead
    ro_fc2 = np.zeros((9, 8), np.float32)
    ro_fc2[0:8] = fc2_w.T  # fc2_w [1,8]
    ro_fc2[8] = fc2_b

    return dict(mh=mh, ro1x=ro1x, ro1h=ro1h,
                ro2h1=ro2h1, ro2h2=ro2h2, ro_fc1=ro_fc1, ro_fc2=ro_fc2)


# ---------------------------------------------------------------------------
# device kernel
# ---------------------------------------------------------------------------

def _build(t_steps=T, fut=FUT, bc=BC, loop_end=None):
    # loop_end < t_steps runs fewer recurrence steps with identical program
    # size and I/O — for isolating device time via wall-clock deltas (output
    # is mathematically meaningless in that mode)
    assert t_steps % U == 0 and t_steps >= 2 * U
    loop_end = t_steps if loop_end is None else loop_end
    assert loop_end % U == 0 and U <= loop_end <= t_steps

    nc = bacc.Bacc("TRN2", target_bir_lowering=False)

    # x chunk-major: xp[u, c*bc + b] = x[b, c*U + u] — a 32-step chunk is a
    # [U, bc] DMA slice spread over U partitions (fast; a single-partition
    # 32KB transfer is ~13us on the per-partition port)
    # the recurrence matmul path (weights, h, x) runs in bf16 — fp32 matmuls
    # measure ~808ns vs ~bf16 ~2-4x less; psum/elementwise/c-state stay fp32
    xp_d = nc.dram_tensor("xp", [U, (t_steps // U) * bc], BF16, kind="ExternalInput")
    ones_d = nc.dram_tensor("onesrow", [1, bc], F32, kind="ExternalInput")
    onesb_d = nc.dram_tensor("onesrow_bf", [1, bc], BF16, kind="ExternalInput")
    w_d = {}
    for name, shape in [("ro1x", [1, 64]), ("ro1h", [17, 64]),
                        ("ro2h1", [17, 64]), ("ro2h2", [16, 64]),
                        ("ro_fc1", [17, 8]), ("ro_fc2", [9, 8])]:
        w_d[name] = nc.dram_tensor(name, shape, F32, kind="ExternalInput")
    mh_d = nc.dram_tensor("mh", [34, 128], BF16, kind="ExternalInput")
    out_d = nc.dram_tensor("out", [fut + 1, bc], F32, kind="ExternalOutput")

    with tile.TileContext(nc) as tc:
        consts = tc.alloc_tile_pool(name="consts", bufs=1)
        states = tc.alloc_tile_pool(name="states", bufs=1)
        work = tc.alloc_tile_pool(name="work", bufs=3)
        xst = tc.alloc_tile_pool(name="xst", bufs=2)
        # main psum (4 banks) and rollout psum (4 banks) stay disjoint for the
        # whole kernel (PE-write/engine-read same-bank hazard across pools)
        psum = tc.alloc_tile_pool(name="psum", bufs=1, space="PSUM")
        psro = tc.alloc_tile_pool(name="psro", bufs=1, space="PSUM")

        w_sb = {}
        for name, t_d in w_d.items():
            w_sb[name] = consts.tile(list(t_d.shape), F32, tag=name, name=name)
            nc.sync.dma_start(out=w_sb[name], in_=t_d[:])
        w_sb["mh"] = consts.tile([34, 128], BF16, tag="mh", name="mh")
        nc.sync.dma_start(out=w_sb["mh"], in_=mh_d[:])

        # rotating rhs buffers: [h1(0:16); h2(16:32); ones(32); x(33)]
        hx = []
        for q in range(2):
            hq = states.tile([34, bc], BF16, tag=f"hx{q}", name=f"hx{q}")
            nc.vector.memset(hq, 0.0)
            nc.sync.dma_start(out=hq[32:33, :], in_=onesb_d[:])
            hx.append(hq)
        cc = states.tile([32, bc], F32, tag="cc")   # [c1; c2]
        nc.vector.memset(cc, 0.0)

        wmh = w_sb["mh"]

        # rollout state tiles (declared early; layer-1 snapshots are taken
        # between main-loop steps T-1 and T)
        h1e = states.tile([17, bc], F32, tag="h1e")  # h1 | ones
        h2e = states.tile([17, bc], F32, tag="h2e")  # h2 | ones
        rc1 = states.tile([16, bc], F32, tag="rc1")
        rc2 = states.tile([16, bc], F32, tag="rc2")

        # ---------------- main teacher-forced loop ----------------
        # psum gate blocks: 0=f, 1=i, 2=o, 3=g (per _ORDER); sif: 0=f, 1=i, 2=o
        def step(par, xnext=None, first=False):
            """One recurrence step. par = j%2. x_j was DMA-prefilled into
            row 33 of cur; xnext = (tile, slice) to prefill x_{j+2} into
            cur's row 33 once this step's matmuls have read it."""
            cur = hx[par]
            nxt = hx[1 - par]
            # one psum tile per gate: separate tiles keep the dependency
            # granularity per-matmul, so each sigmoid starts right after its
            # own matmul instead of after all four
            g = [psum.tile([32, bc], F32, tag=f"g{k}", name=f"g{k}")
                 for k in range(4)]
            for k in (0, 1, 3, 2):  # f, i, g, o — the o gate is needed last
                nc.tensor.matmul(g[k], wmh[:, 32 * k:32 * k + 32], cur,
                                 start=True, stop=True)
            if xnext is not None:
                nc.sync.dma_start(out=cur[33:34, :], in_=xnext)

            sif = work.tile([32, 3, bc], F32, tag="sif")
            tg = work.tile([32, bc], F32, tag="tg")
            # per-gate ACTs so the c-update chain starts after matmul 1 of 4
            nc.scalar.activation(sif[:, 0, :], g[0], AF.Sigmoid)
            nc.scalar.activation(sif[:, 1, :], g[1], AF.Sigmoid)
            nc.scalar.activation(tg, g[3], AF.Tanh)

            # first step: layer-1 half only (layer-2 gates are not yet valid).
            # final step: full window (base-16 slices are illegal); the
            # layer-1 results of that step are junk but harmless — rc1/h1e
            # snapshot layer-1 state before its writes land.
            s0, s1 = (0, 16) if first else (0, 32)
            m1 = work.tile([32, bc], F32, tag="m1")
            m2 = work.tile([32, bc], F32, tag="m2")
            tc_ = work.tile([32, bc], F32, tag="tc")
            nc.vector.tensor_mul(m1[s0:s1], sif[s0:s1, 0, :], cc[s0:s1])
            nc.vector.tensor_mul(m2[s0:s1], sif[s0:s1, 1, :], tg[s0:s1])
            nc.vector.tensor_add(cc[s0:s1], m1[s0:s1], m2[s0:s1])
            nc.scalar.activation(sif[:, 2, :], g[2], AF.Sigmoid)
            nc.scalar.activation(tc_[s0:s1], cc[s0:s1], AF.Tanh)
            nc.vector.tensor_mul(nxt[s0:s1], sif[s0:s1, 2, :], tc_[s0:s1])

        def xsl(xt, u):
            return xt[u:u + 1, :]

        def run_chunk(xt, first=False):
            """U steps consuming x rows 0..U-1 of chunk tile xt; x for the
            first two steps must already be in the hx row-33 slots."""
            for u in range(U):
                step(u % 2, xnext=(xsl(xt, u + 2) if u + 2 < U else None),
                     first=(first and u == 0))

        # prologue: steps 0..U-1 unrolled (step 0 is the layer1-only special)
        xpro = xst.tile([U, bc], BF16, tag="xch")
        nc.sync.dma_start(out=xpro, in_=xp_d[:, 0:bc])
        nc.sync.dma_start(out=hx[0][33:34, :], in_=xsl(xpro, 0))
        nc.sync.dma_start(out=hx[1][33:34, :], in_=xsl(xpro, 1))
        run_chunk(xpro, first=True)

        # steps U..loop_end-1 in a hardware loop, U per iteration.
        # loop variable is the chunk's element offset within an xp row.
        with tc.For_i(bc, (loop_end // U) * bc, bc) as coff:
            xch = xst.tile([U, bc], BF16, tag="xch")
            nc.sync.dma_start(out=xch, in_=xp_d[:, ds(coff, bc)])
            nc.sync.dma_start(out=hx[0][33:34, :], in_=xsl(xch, 0))
            nc.sync.dma_start(out=hx[1][33:34, :], in_=xsl(xch, 1))
            run_chunk(xch)

        # snapshot layer-1 state before the final (layer2-only) step clobbers it
        nc.scalar.copy(h1e[0:16], hx[0][0:16])   # h1(T-1)
        nc.scalar.copy(rc1, cc[0:16])            # c1(T-1)
        step(0)  # layer2-only finish; row 33 holds stale x, layer-1 junk is unused

        # ---------------- rollout ----------------
        ot = states.tile([1, bc], F32, tag="ot")     # current head output
        zt = states.tile([9, bc], F32, tag="zt")     # leaky(fc1) | ones
        out_sb = states.tile([1, fut + 1, bc], F32, tag="out_sb")
        # ones rows (memset can't start at partition 16/8 — DMA from DRAM)
        nc.sync.dma_start(out=h1e[16:17, :], in_=ones_d[:])
        nc.sync.dma_start(out=h2e[16:17, :], in_=ones_d[:])
        nc.sync.dma_start(out=zt[8:9, :], in_=ones_d[:])

        # h2(T-1): repartition via DMA (bf16), then cast to the fp32 rollout tile
        h2tmp = states.tile([16, bc], BF16, tag="h2tmp")
        nc.sync.dma_start(out=h2tmp[:], in_=hx[1][16:32, :])
        nc.scalar.copy(h2e[0:16, :], h2tmp[:])
        nc.sync.dma_start(out=rc2[:], in_=cc[16:32, :])

        def head(r):
            z = psro.tile([8, bc], F32, tag="roz")
            nc.tensor.matmul(z, w_sb["ro_fc1"], h2e, start=True, stop=True)
            zs = work.tile([8, bc], F32, tag="zs")
            nc.scalar.mul(zs, z, 0.2)
            nc.vector.tensor_max(zt[0:8], z, zs)  # leaky relu 0.2
            o_ps = psro.tile([8, bc], F32, tag="roo")
            nc.tensor.matmul(o_ps, w_sb["ro_fc2"], zt, start=True, stop=True)
            nc.scalar.copy(out_sb[:, r, :], o_ps[0:1])
            if r <= fut - 1:
                nc.scalar.copy(ot, o_ps[0:1])

        def ro_cell(mms, rc, h_out):
            gr = psro.tile([16, 4, bc], F32, tag="rog")
            for k in range(4):
                for i, (lhsT, rhs) in enumerate(mms):
                    nc.tensor.matmul(gr[:, k, :], lhsT[:, 16 * k:16 * k + 16], rhs,
                                     start=(i == 0), stop=(i == len(mms) - 1))
            sifr = work.tile([16, 3, bc], F32, tag="sifr")
            nc.scalar.activation(sifr, gr[:, 0:3, :], AF.Sigmoid)
            tgr = work.tile([16, bc], F32, tag="tgr")
            nc.scalar.activation(tgr, gr[:, 3, :], AF.Tanh)
            a1 = work.tile([16, bc], F32, tag="a1")
            a2 = work.tile([16, bc], F32, tag="a2")
            tcr = work.tile([16, bc], F32, tag="tcr")
            nc.vector.tensor_mul(a1, sifr[:, 0, :], rc)   # f-hat (gate order f,i,o,g)
            nc.vector.tensor_mul(a2, sifr[:, 1, :], tgr)  # i-hat
            nc.vector.tensor_add(rc, a1, a2)
            nc.scalar.activation(tcr, rc, AF.Tanh)
            nc.vector.tensor_mul(h_out, sifr[:, 2, :], tcr)

        head(0)
        for r in range(fut):
            ro_cell([(w_sb["ro1x"], ot), (w_sb["ro1h"], h1e)], rc1, h1e[0:16])
            ro_cell([(w_sb["ro2h1"], h1e), (w_sb["ro2h2"], h2e[0:16])], rc2, h2e[0:16])
            head(r + 1)

        # keep the partition dim in the AP — integer-indexing it away breaks
        # Tile's subtile dependency tracking (the DMA then reads stale data)
        nc.sync.dma_start(out=out_d[:].rearrange("(o f) b -> o f b", o=1), in_=out_sb)

        for p_ in (psro, psum, xst, work, states, consts):
            p_.release()

    if not nc.is_finalized():
        nc.finalize()
    return nc


# ---------------------------------------------------------------------------
# host wrapper: cached build + cached jitted PJRT runner
# ---------------------------------------------------------------------------

_CACHED = {}


def _get_nc(t_steps, fut, bc, loop_end=None):
    key = (t_steps, fut, bc, loop_end)
    if key not in _CACHED:
        _CACHED[key] = _build(t_steps, fut, bc, loop_end=loop_end)
    return _CACHED[key]


_RUNNERS = {}


def _get_runner(nc, n_cores, nreps=1):
    """Build (once per nc) a jitted shard_map runner for the program, so
    repeat calls don't re-trace/re-compile the executable. nreps > 1 chains
    the NEFF execution on-device nreps times (each exec's outputs feed the
    next one's donated output buffers) — for timing isolation."""
    key = (id(nc), nreps)
    if key in _RUNNERS:
        return _RUNNERS[key]
    import jax
    from jax.sharding import Mesh, PartitionSpec
    from jax.experimental.shard_map import shard_map

    bass2jax.install_neuronx_cc_hook()

    partition_name = nc.partition_id_tensor.name if nc.partition_id_tensor else None
    in_names, out_names, out_avals = [], [], []
    for alloc in nc.m.functions[0].allocations:
        if not isinstance(alloc, mybir.MemoryLocationSet):
            continue
        name = alloc.memorylocations[0].name
        if alloc.kind == "ExternalInput":
            if name != partition_name:
                in_names.append(name)
        elif alloc.kind == "ExternalOutput":
            assert alloc.tensor_shape is not None and alloc.dtype is not None
            out_names.append(name)
            out_avals.append(jax.core.ShapedArray(
                tuple(alloc.tensor_shape), mybir.dt.np(alloc.dtype)))
    n_params = len(in_names)
    all_names = list(in_names) + list(out_names)
    if partition_name is not None:
        all_names.append(partition_name)
    donate = tuple(range(n_params, n_params + len(out_names)))

    def _body(*args):
        ins = list(args[:n_params])
        outs = list(args[n_params:])
        for _ in range(nreps):
            operands = ins + outs
            if partition_name is not None:
                operands.append(bass2jax.partition_id_tensor())
            outs = list(bass2jax._bass_exec_p.bind(
                *operands,
                out_avals=tuple(out_avals),
                in_names=tuple(all_names),
                out_names=tuple(out_names),
                lowering_input_output_aliases=(),
                sim_require_finite=True,
                sim_require_nnan=True,
                nc=nc,
            ))
        return tuple(outs)

    devices = jax.devices()[:n_cores]
    mesh = Mesh(np.asarray(devices), ("core",))
    nspecs = n_params + len(out_names)
    fn = jax.jit(
        shard_map(_body, mesh=mesh,
                  in_specs=(PartitionSpec("core"),) * nspecs,
                  out_specs=(PartitionSpec("core"),) * len(out_names),
                  check_rep=False),
        donate_argnums=donate, keep_unused=True)

    out_zero = [(tuple(a.shape), a.dtype) for a in out_avals]
    _RUNNERS[key] = (fn, list(in_names), list(out_names), out_zero)
    return _RUNNERS[key]


def _run(nc, in_maps, n_cores, nreps=1):
    fn, in_names, out_names, out_zero = _get_runner(nc, n_cores, nreps)
    concat_in = [np.concatenate([np.asarray(m[name]) for m in in_maps], axis=0)
                 for name in in_names]
    concat_zeros = [np.zeros((n_cores * s[0], *s[1:]), d) for s, d in out_zero]
    out_arrs = fn(*concat_in, *concat_zeros)
    res = []
    for c in range(n_cores):
        res.append({name: np.asarray(out_arrs[i]).reshape(
            n_cores, -1, *out_arrs[i].shape[1:])[c].reshape(out_arrs[i].shape[0] // n_cores, *out_arrs[i].shape[1:])
            for i, name in enumerate(out_names)})
    return res


def _make_in_maps(x, W_ih1, W_hh1, b_ih1, b_hh1, W_ih2, W_hh2, b_ih2, b_hh2,
                  fc1_w, fc1_b, fc2_w, fc2_b, t_steps, bc):
    import ml_dtypes
    bf16 = ml_dtypes.bfloat16
    w = _pack_weights(np.asarray(W_ih1, np.float32), np.asarray(W_hh1, np.float32),
                      np.asarray(b_ih1, np.float32), np.asarray(b_hh1, np.float32),
                      np.asarray(W_ih2, np.float32), np.asarray(W_hh2, np.float32),
                      np.asarray(b_ih2, np.float32), np.asarray(b_hh2, np.float32),
                      np.asarray(fc1_w, np.float32), np.asarray(fc1_b, np.float32),
                      np.asarray(fc2_w, np.float32), np.asarray(fc2_b, np.float32))
    w["mh"] = w["mh"].astype(bf16)
    w["onesrow"] = np.ones((1, bc), np.float32)
    w["onesrow_bf"] = np.ones((1, bc), bf16)
    in_maps = []
    for c in range(NCORES):
        m = dict(w)
        # chunk-major: xp[u, cb*bc + b] = x[b, cb*U + u]
        xc = x[c * bc:(c + 1) * bc, :t_steps].T.reshape(t_steps // U, U, bc)
        m["xp"] = np.ascontiguousarray(xc.transpose(1, 0, 2).reshape(U, -1)).astype(bf16)
        in_maps.append(m)
    return in_maps


def kernel(x, W_ih1, W_hh1, b_ih1, b_hh1, W_ih2, W_hh2, b_ih2, b_hh2,
           fc1_w, fc1_b, fc2_w, fc2_b, future, _t_steps=None, _loop_end=None,
           _nreps=1):
    x = np.asarray(x, np.float32)
    fut = int(future)
    t_steps = int(_t_steps or x.shape[1])
    bc = x.shape[0] // NCORES

    nc = _get_nc(t_steps, fut, bc, _loop_end)
    in_maps = _make_in_maps(x, W_ih1, W_hh1, b_ih1, b_hh1, W_ih2, W_hh2,
                            b_ih2, b_hh2, fc1_w, fc1_b, fc2_w, fc2_b, t_steps, bc)

    res = _run(nc, in_maps, NCORES, nreps=_nreps)
    outs = [res[c]["out"] for c in range(NCORES)]  # each [fut+1, bc]
    full = np.concatenate(outs, axis=1).T  # [B, fut+1]
    return np.ascontiguousarray(full.astype(np.float32))
